# revision 5
# baseline (speedup 1.0000x reference)
"""LIF neuron scan kernel for Trainium2 (8 NeuronCores, SPMD).

Reference semantics (per element, scan over T):
    H[t] = V[t-1] - (V[t-1] - 0.5)/2 + x[t]
    S[t] = (H[t] >= 1.0)
    V[t] = S[t] ? 0.5 : H[t]

Kernel formulation with g[t] = H[t] - 0.5 (bit-identical in fp32):
    g[0]   = x[0]
    S[t]   = (g[t] >= 0.5)
    g[t+1] = 0.5 * (g[t] * [g[t] < 0.5]) + x[t+1]

Engine mapping per timestep:
    DVE:  z = (g is_lt 0.5) mult g          (scalar_tensor_tensor)
    DVE:  g' = (z mult 0.5) add x[t+1]      (scalar_tensor_tensor)
    ACT:  s = Sign(g - 0.5) -> int8 {-1,+1} (spike indicator, exact)
    DMA:  x[t+1] in (512 KiB), s[t] out (128 KiB int8)

The serial dependency chain lives entirely on DVE (2 ops/step); the
spike output is produced off-chain on the Activation engine as an exact
+-1 sign code in int8 (1.0/-1.0 -> int8 is exact under any rounding
mode), and decoded to 0.0/1.0 float32 on the host during unsharding.
Data-parallel over (B*N) across the 8 cores; no cross-device
communication.
"""

import sys

import numpy as np

if "/opt/trn_rl_repo" not in sys.path:
    sys.path.insert(0, "/opt/trn_rl_repo")

import bass_rust
import concourse.bass as bass
import concourse.mybir as mybir
import concourse.tile as tile
from concourse.bass_utils import run_bass_kernel_spmd

T, B, N = 64, 32, 32768
NCORES = 8
BN = B * N
PER = BN // NCORES  # 131072 elements per core per timestep
P = 128
F = PER // P  # 1024

_CACHE = {}


def _split_excess_waits(nc: bass.Bass, limit: int = 1) -> None:
    """This walrus codegen rejects any instruction carrying more than one
    sync-wait command.  Move the excess waits onto same-engine NoOps
    inserted immediately before the offending instruction — semantically
    identical, the engine just performs the waits one slot earlier in its
    own stream (one wait per NoOp)."""
    n = 0
    for f in nc.m.functions:
        for blk in f.blocks:
            insts = blk.instructions
            out = []
            for inst in insts:
                si = inst.sync_info
                if si is not None and len(si.on_wait) > limit:
                    waits = list(si.on_wait)
                    excess, keep = waits[:-limit], waits[-limit:]
                    for w in excess:
                        nop = bass_rust.InstNoOp(name=f"I-waitnop-{n}")
                        n += 1
                        nop.engine = inst.engine
                        nop.sync_info = bass_rust.SyncInfo(
                            on_wait=[w], on_update=[]
                        )
                        out.append(nop)
                    si.on_wait = keep
                out.append(inst)
            blk.instructions = out


def build_nc() -> bass.Bass:
    nc = bass.Bass()
    f32 = mybir.dt.float32
    i8 = mybir.dt.int8
    x = nc.dram_tensor("x", [T, P, F], f32, kind="ExternalInput")
    s = nc.dram_tensor("s", [T, P, F], i8, kind="ExternalOutput")

    alu = mybir.AluOpType
    with tile.TileContext(nc) as tc:
        with (
            tc.tile_pool(name="xin", bufs=10) as xpool,
            tc.tile_pool(name="gz", bufs=3) as gpool,
            tc.tile_pool(name="sout", bufs=8) as spool,
            tc.tile_pool(name="consts", bufs=1) as cpool,
        ):
            # -0.5 bias for the Sign activation, dep-tracked by the tile
            # framework (avoids a raw const + all-engine barrier).
            neghalf = cpool.tile([P, 1], f32)
            nc.vector.memset(neghalf[:], -0.5)
            nc.const_aps.aps[(f32, -0.5)] = neghalf[:]

            xn = xpool.tile([P, F], f32)
            nc.sync.dma_start(xn[:], x[0])
            g = xn  # g[0] = x[0]
            for t in range(T):
                st = spool.tile([P, F], i8)
                nc.scalar.activation(
                    st[:], g[:], mybir.ActivationFunctionType.Sign, bias=-0.5
                )
                nc.sync.dma_start(s[t], st[:])
                if t + 1 < T:
                    z = gpool.tile([P, F], f32, tag="z", bufs=2)
                    nc.vector.scalar_tensor_tensor(
                        z[:], g[:], 0.5, g[:], alu.is_lt, alu.mult
                    )
                    xn = xpool.tile([P, F], f32)
                    nc.sync.dma_start(xn[:], x[t + 1])
                    g = gpool.tile([P, F], f32, tag="g", bufs=3)
                    nc.vector.scalar_tensor_tensor(
                        g[:], z[:], 0.5, xn[:], alu.mult, alu.add
                    )
    _split_excess_waits(nc)
    return nc


def _get_nc() -> bass.Bass:
    if "nc" not in _CACHE:
        _CACHE["nc"] = build_nc()
    return _CACHE["nc"]


def kernel(x: np.ndarray, **run_kwargs):
    x = np.asarray(x)
    assert x.shape == (T, B, N), x.shape
    assert x.dtype == np.float32, x.dtype
    xf = x.reshape(T, BN)
    in_maps = [
        {"x": np.ascontiguousarray(xf[:, k * PER : (k + 1) * PER]).reshape(T, P, F)}
        for k in range(NCORES)
    ]
    res = run_bass_kernel_spmd(_get_nc(), in_maps, list(range(NCORES)), **run_kwargs)
    out = np.empty((T, BN), dtype=np.float32)
    for k in range(NCORES):
        sk = res.results[k]["s"].reshape(T, PER)
        out[:, k * PER : (k + 1) * PER] = (sk > 0).astype(np.float32)
    out = out.reshape(T, B, N)
    if run_kwargs:
        return out, res
    return out


# revision 7
# speedup vs baseline: 1.0076x; 1.0076x over previous
"""LIF neuron scan kernel for Trainium2 (8 NeuronCores, SPMD).

Reference semantics (per element, scan over T):
    H[t] = V[t-1] - (V[t-1] - 0.5)/2 + x[t]
    S[t] = (H[t] >= 1.0)
    V[t] = S[t] ? 0.5 : H[t]

Kernel formulation with g[t] = H[t] - 0.5 (bit-identical in fp32):
    g[0]   = x[0]
    S[t]   = (g[t] >= 0.5)
    g[t+1] = 0.5 * (g[t] * [g[t] < 0.5]) + x[t+1]

Engine mapping per timestep:
    DVE:  z = (g is_lt 0.5) mult g          (scalar_tensor_tensor)
    DVE:  g' = (z mult 0.5) add x[t+1]      (scalar_tensor_tensor)
    ACT:  s = Sign(g - 0.5) -> int8 {-1,+1} (spike indicator, exact)
    DMA:  x[t+1] in (512 KiB), s[t] out (128 KiB int8)

The serial dependency chain lives entirely on DVE (2 ops/step); the
spike output is produced off-chain on the Activation engine as an exact
+-1 sign code in int8 (1.0/-1.0 -> int8 is exact under any rounding
mode), and decoded to 0.0/1.0 float32 on the host during unsharding.
Data-parallel over (B*N) across the 8 cores; no cross-device
communication.
"""

import sys

import numpy as np

if "/opt/trn_rl_repo" not in sys.path:
    sys.path.insert(0, "/opt/trn_rl_repo")

import bass_rust
import concourse.bass as bass
import concourse.mybir as mybir
import concourse.tile as tile
from concourse.bass_utils import run_bass_kernel_spmd

T, B, N = 64, 32, 32768
NCORES = 8
BN = B * N
PER = BN // NCORES  # 131072 elements per core per timestep
P = 128
F = PER // P  # 1024

_CACHE = {}


def _split_excess_waits(nc: bass.Bass, limit: int = 1) -> None:
    """This walrus codegen rejects any instruction carrying more than one
    sync-wait command.  Move the excess waits onto same-engine NoOps
    inserted immediately before the offending instruction — semantically
    identical, the engine just performs the waits one slot earlier in its
    own stream (one wait per NoOp)."""
    n = 0
    for f in nc.m.functions:
        for blk in f.blocks:
            insts = blk.instructions
            out = []
            for inst in insts:
                si = inst.sync_info
                if si is not None and len(si.on_wait) > limit:
                    waits = list(si.on_wait)
                    excess, keep = waits[:-limit], waits[-limit:]
                    for w in excess:
                        nop = bass_rust.InstNoOp(name=f"I-waitnop-{n}")
                        n += 1
                        nop.engine = inst.engine
                        nop.sync_info = bass_rust.SyncInfo(
                            on_wait=[w], on_update=[]
                        )
                        out.append(nop)
                    si.on_wait = keep
                out.append(inst)
            blk.instructions = out


def build_nc() -> bass.Bass:
    nc = bass.Bass()
    f32 = mybir.dt.float32
    i8 = mybir.dt.int8
    x = nc.dram_tensor("x", [T, P, F], f32, kind="ExternalInput")
    s = nc.dram_tensor("s", [T, P, F], i8, kind="ExternalOutput")

    alu = mybir.AluOpType
    with tile.TileContext(nc) as tc:
        with (
            tc.tile_pool(name="xin", bufs=10) as xpool,
            tc.tile_pool(name="gz", bufs=5) as gpool,
            tc.tile_pool(name="sout", bufs=10) as spool,
            tc.tile_pool(name="consts", bufs=1) as cpool,
        ):
            # -0.5 bias for the Sign activation, dep-tracked by the tile
            # framework (avoids a raw const + all-engine barrier).
            neghalf = cpool.tile([P, 1], f32)
            nc.vector.memset(neghalf[:], -0.5)
            nc.const_aps.aps[(f32, -0.5)] = neghalf[:]

            xn = xpool.tile([P, F], f32)
            nc.sync.dma_start(xn[:], x[0])
            g = xn  # g[0] = x[0]
            for t in range(T):
                st = spool.tile([P, F], i8)
                nc.scalar.activation(
                    st[:], g[:], mybir.ActivationFunctionType.Sign, bias=-0.5
                )
                nc.sync.dma_start(s[t], st[:])
                if t + 1 < T:
                    z = gpool.tile([P, F], f32, tag="z", bufs=4)
                    nc.vector.scalar_tensor_tensor(
                        z[:], g[:], 0.5, g[:], alu.is_lt, alu.mult
                    )
                    xn = xpool.tile([P, F], f32)
                    nc.sync.dma_start(xn[:], x[t + 1])
                    g = gpool.tile([P, F], f32, tag="g", bufs=5)
                    nc.vector.scalar_tensor_tensor(
                        g[:], z[:], 0.5, xn[:], alu.mult, alu.add
                    )
    _split_excess_waits(nc)
    return nc


def _get_nc() -> bass.Bass:
    if "nc" not in _CACHE:
        _CACHE["nc"] = build_nc()
    return _CACHE["nc"]


def kernel(x: np.ndarray, **run_kwargs):
    x = np.asarray(x)
    assert x.shape == (T, B, N), x.shape
    assert x.dtype == np.float32, x.dtype
    xf = x.reshape(T, BN)
    in_maps = [
        {"x": np.ascontiguousarray(xf[:, k * PER : (k + 1) * PER]).reshape(T, P, F)}
        for k in range(NCORES)
    ]
    res = run_bass_kernel_spmd(_get_nc(), in_maps, list(range(NCORES)), **run_kwargs)
    out = np.empty((T, BN), dtype=np.float32)
    for k in range(NCORES):
        sk = res.results[k]["s"].reshape(T, PER)
        out[:, k * PER : (k + 1) * PER] = (sk > 0).astype(np.float32)
    out = out.reshape(T, B, N)
    if run_kwargs:
        return out, res
    return out
